# revision 22
# baseline (speedup 1.0000x reference)
"""Weighted 2D cross-entropy (BCE-over-classes) loss on 8 Trainium2 cores.

Math (matches the reference):
  t in [0,19); pos = t>0, neg = t==0 (all pixels are pos or neg; mask == 1)
  S(i) = sum_c bce(i,c) = -lnR(i)
     lnR(i) = A(i) + B(i)
     A(i)   = sum_c ln(1-p_c(i))
     B(i)   = ln(p_t(i)) - ln(1-p_t(i)) = ln(e^{-lsel(i)} - 1),  lsel = ln(1-p_t)
  loss = ( (NEG/TOT)*S_pos_sum + (POS/TOT)*S_neg_sum ) / (TOT*C)

Per-core (core k <- batch element k, pure data parallel), FOUR quarter-passes
over pixel quarters [128, 1024].  A quarter's PSUM accumulators are four
single-bank [128, 512] half-tiles (A and lsel, each split in half), so two
quarters ping-pong in PSUM (8 banks total): while quarter q's tail
(Exp/Ln/STT chain on ACT+DVE) drains its banks, the PE already streams
quarter q+1's matmuls, and each 512-wide tail chunk depends only on its own
half's matmuls (whole-tile deps would chain it to the last matmul).

Schedule, tuned against the 1-wait-per-instruction TRN2 sync structs (a
multi-wait op gets its extra waits split onto PRECEDING ops of the same
in-order engine, stalling them):
  - quarter 0 opens with two single-DMA 512-wide class-18 chunks: their Ln
    needs one wait, so the ACT_TABLE_LOAD (first ACT-queue entry) stays
    wait-free and runs during the DMA ramp, and the first Ln starts on
    0.25MB instead of 1MB of data.
  - quarters 0-2 then process classes in PAIRS: one Ln / one mult covers
    [128, 2048], halving the ~294ns fixed per-op ACT/DVE overhead (ACT
    otherwise paces the DMA-bound stream).  Each pair is TWO plain 2D DMAs
    into one tile (a single 3D [p, c, f] DMA costs ~2.2us of serial
    descriptor-generation on the sync sequencer vs ~0.6us x2).
  - quarter 3 processes classes SINGLY (single-DMA 0.5MB units, class 18 in
    two 512 chunks): after the last DMA byte only a small unit remains in
    flight, so the end-drain is short.
  - tail per half: expm=Exp(-lsel); B=Ln(expm-1) (fused -1 bias via a
    [128,1] const column); lnR=B+A via STT with accum_out; pos-masked sum
    via a second STT accum.  tail(q) is deferred and its 8 ops spread one
    per unit across quarter q+1, each emitted >=1 unit after its producer
    so no in-order engine stream ever stalls behind it.
Target is converted to bf16 on HOST (1MB instead of 2MB int32 DMA, no
on-chip CAST, and the first predict tile lands sooner).
Activation tables are pinned to natural_log_exp_and_others (holds both
ln and exp) -- otherwise bacc's table-load pass alternates between the
ln-only and exp-only sets, paying ~1.3us per reload.
Counts (pos/neg) are computed on host from the int target directly.
Per-core output is the raw [128, 16] per-partition stats; the final
partition reduce + 8-way combine happens on host in float64.
"""

from contextlib import ExitStack

import numpy as np

import concourse.bass as bass
import concourse.mybir as mybir
import concourse.tile as tile
from concourse import bacc
from concourse.bass_utils import run_bass_kernel_spmd

# problem shape (hardcoded per harness contract)
N, C, H, W = 8, 19, 512, 1024
PIX = H * W          # 524288 pixels per core
P = 128              # partitions
FCOLS = PIX // P     # 4096 free columns when pixels laid out [128, 4096]
QW = FCOLS // 4      # 1024: quarter width
HQW = QW // 2        # 512: half-quarter (PSUM bank / matmul / tail width)
NPAIR = C // 2       # 9 class pairs; class 18 is the unpaired class
N_CORES = 8
NSTAT = 16           # stats columns in the [128, 16] output

DT = mybir.dt

# stats column layout ([128, 16] f32; host folds):
#   2q+h     : sum lnR      for quarter q, half h
#   8+2q+h   : sum pos*lnR  for quarter q, half h
COL_LNR = 0
COL_POSLNR = 8

_ACT_TABLES_PATCHED = False


def _pin_act_table_set():
    """Restrict Ln/Exp to the natural_log_exp_and_others set so bacc's
    table-load pass emits a single ACT_TABLE_LOAD instead of thrashing
    between the ln-only and exp-only sets (~1.3us per reload).  Set
    indices must stay aligned with act_info.json, so every set entry is
    kept -- only the Ln/Exp membership of the other sets is dropped."""
    global _ACT_TABLES_PATCHED
    if _ACT_TABLES_PATCHED:
        return
    import concourse.bacc as bacc_mod

    orig = bacc_mod.get_activation_tables
    ln_exp = {mybir.ActivationFunctionType.Ln, mybir.ActivationFunctionType.Exp}

    def patched(arch):
        tables = orig(arch)
        return {
            name: (fns if name == "natural_log_exp_and_others" else fns - ln_exp)
            for name, fns in tables.items()
        }

    bacc_mod.get_activation_tables = patched
    _ACT_TABLES_PATCHED = True


def build_kernel() -> bass.Bass:
    _pin_act_table_set()

    # Bacc (not raw Bass): its compile() pipeline runs
    # generate_event_semaphores, which splits multi-sem waits to satisfy the
    # 1-wait-per-instruction TRN2 sync structs -- raw Bass modules with
    # Tile-emitted multi-waits fail walrus codegen.
    nc = bacc.Bacc("TRN2")

    predict = nc.declare_dram_parameter("predict", [C, PIX], DT.float32, isOutput=False)
    target = nc.declare_dram_parameter("target", [P, FCOLS], DT.bfloat16, isOutput=False)
    idn = nc.declare_dram_parameter("idn", [P, P], DT.bfloat16, isOutput=False)
    out = nc.declare_dram_parameter("out", [P, NSTAT], DT.float32, isOutput=True)

    pred_r = predict.rearrange("c (p f) -> c p f", p=P)  # [19, 128, 4096]

    with tile.TileContext(nc) as tc, ExitStack() as ctx:
        const = ctx.enter_context(tc.tile_pool(name="const", bufs=1))
        # ps bufs=8 aligns slot reuse with the global DMA->DMAHW-proc
        # round-robin (8 procs), so the WAW on the old writer is same-proc
        # FIFO order and Tile emits no cross-queue wait; 8 units = 4MB of
        # DMA lookahead
        ps_pool = ctx.enter_context(tc.tile_pool(name="ps", bufs=8))
        lms_pool = ctx.enter_context(tc.tile_pool(name="lms", bufs=4))
        eqs_pool = ctx.enter_context(tc.tile_pool(name="eqs", bufs=4))
        tail_pool = ctx.enter_context(tc.tile_pool(name="tail", bufs=2))
        psAa_pool = ctx.enter_context(tc.tile_pool(name="psAa", bufs=2, space="PSUM"))
        psAb_pool = ctx.enter_context(tc.tile_pool(name="psAb", bufs=2, space="PSUM"))
        psLa_pool = ctx.enter_context(tc.tile_pool(name="psLa", bufs=2, space="PSUM"))
        psLb_pool = ctx.enter_context(tc.tile_pool(name="psLb", bufs=2, space="PSUM"))

        t_bf = const.tile([P, FCOLS], DT.bfloat16, tag="tb")
        # quarter 0 of target first so the q0 eq chain is ready before p0
        nc.sync.dma_start(out=t_bf[:, 0:QW], in_=target[:, 0:QW])

        idn_sb = const.tile([P, P], DT.bfloat16, tag="idn")
        stats = const.tile([P, NSTAT], DT.float32, tag="stats")
        # per-partition -1.0 bias column for the fused Ln(expm - 1) tail
        negone = const.tile([P, 1], DT.float32, tag="negone")

        state = {"n_dma": 0}

        def count_dma():
            # constants queue behind the first data DMA; the bulk of target
            # queues behind the second -- the q0 pipeline primes first
            state["n_dma"] += 1
            if state["n_dma"] == 1:
                nc.sync.dma_start(out=idn_sb[:], in_=idn[:])
                nc.vector.memset(stats[:], 0.0)
                nc.vector.memset(negone[:], -1.0)
            elif state["n_dma"] == 2:
                nc.sync.dma_start(out=t_bf[:, QW:], in_=target[:, QW:])

        def emit_single(q, c, off, width, halves, start, stop):
            # one class's [off, off+width) slice of quarter q: single DMA,
            # Ln, eq, mask-mult, then per-512 matmuls into the half tiles
            qbase = q * QW
            csl = slice(qbase + off, qbase + off + width)
            p_s = ps_pool.tile([P, QW], DT.float32, tag="ps")
            nc.sync.dma_start(out=p_s[:, :width], in_=pred_r[c, :, csl])
            count_dma()
            lm = lms_pool.tile([P, 2 * QW], DT.bfloat16, tag="lms")
            nc.scalar.activation(
                out=lm[:, :width],
                in_=p_s[:, :width],
                func=mybir.ActivationFunctionType.Ln,
                bias=1.0,
                scale=-1.0,
            )
            eq = eqs_pool.tile([P, QW], DT.bfloat16, tag="eqs")
            nc.vector.tensor_scalar(
                out=eq[:, :width],
                in0=t_bf[:, csl],
                scalar1=float(c),
                scalar2=None,
                op0=mybir.AluOpType.is_equal,
            )
            nc.vector.tensor_mul(
                out=lm[:, QW : QW + width],
                in0=eq[:, :width],
                in1=lm[:, :width],
            )
            for s in range(width // HQW):
                h = (off + s * HQW) // HQW
                l_h, a_h = halves[h]
                nc.tensor.matmul(
                    l_h[:, :],
                    lhsT=idn_sb[:],
                    rhs=lm[:, QW + s * HQW : QW + (s + 1) * HQW],
                    start=start,
                    stop=stop,
                )
            for s in range(width // HQW):
                h = (off + s * HQW) // HQW
                l_h, a_h = halves[h]
                nc.tensor.matmul(
                    a_h[:, :],
                    lhsT=idn_sb[:],
                    rhs=lm[:, s * HQW : (s + 1) * HQW],
                    start=start,
                    stop=stop,
                )

        def tail_ops(q, h, l_h, a_h):
            # tail for half h of quarter q: B = Ln(e^{-lsel} - 1) (bias
            # fuses the -1); lnR = B + A; two accumulating STTs.
            # Returned as 4 thunks so the caller can spread them across the
            # NEXT quarter's units: ACT and DVE are in-order engines, so an
            # op must only be emitted once its producer ran >=1 unit
            # earlier, else the whole engine stream stalls behind it.
            col = 2 * q + h
            toff = q * QW + h * HQW
            expm = tail_pool.tile([P, HQW], DT.float32, tag=f"expm{h}")
            bb = tail_pool.tile([P, HQW], DT.float32, tag=f"bb{h}")
            lnr = tail_pool.tile([P, HQW], DT.float32, tag=f"lnr{h}")
            scr = tail_pool.tile([P, HQW], DT.float32, tag=f"scr{h}")

            def op_exp():
                nc.scalar.activation(
                    out=expm[:, :],
                    in_=l_h[:, :],
                    func=mybir.ActivationFunctionType.Exp,
                    scale=-1.0,
                )

            def op_lnb():
                nc.scalar.activation(
                    out=bb[:, :],
                    in_=expm[:, :],
                    func=mybir.ActivationFunctionType.Ln,
                    bias=negone[:],
                )

            def op_lnr():
                nc.vector.scalar_tensor_tensor(
                    out=lnr[:, :],
                    in0=bb[:, :],
                    scalar=0.0,
                    in1=a_h[:, :],
                    op0=mybir.AluOpType.add,
                    op1=mybir.AluOpType.add,
                    accum_out=stats[:, COL_LNR + col : COL_LNR + col + 1],
                )

            def op_scr():
                nc.vector.scalar_tensor_tensor(
                    out=scr[:, :],
                    in0=t_bf[:, toff : toff + HQW],
                    scalar=0.5,
                    in1=lnr[:, :],
                    op0=mybir.AluOpType.is_gt,
                    op1=mybir.AluOpType.mult,
                    accum_out=stats[:, COL_POSLNR + col : COL_POSLNR + col + 1],
                )

            return [op_exp, op_lnb, op_lnr, op_scr]

        # tail(q)'s 8 thunks are spread one per unit across quarter q+1,
        # starting at its second unit
        pending_ops = []

        for q in range(4):
            # PSUM half accumulators for this quarter (ping-pong, 1 bank each)
            halves = []
            for h, (lp, ap) in enumerate(
                ((psLa_pool, psAa_pool), (psLb_pool, psAb_pool))
            ):
                l_h = lp.tile([P, HQW], DT.float32, tag=f"l{h}")
                a_h = ap.tile([P, HQW], DT.float32, tag=f"a{h}")
                halves.append((l_h, a_h))

            def unit_boundary():
                if pending_ops:
                    pending_ops.pop(0)()

            if q == 0:
                # prime: two single-DMA 512 chunks of class 18 open the
                # accumulation, so the first Ln starts on 0.25MB of data
                emit_single(q, C - 1, 0, HQW, halves, start=True, stop=False)
                emit_single(q, C - 1, HQW, HQW, halves, start=True, stop=False)
                for c in range(C - 1):
                    if c >= 1:
                        unit_boundary()
                    emit_single(q, c, 0, QW, halves, start=False,
                                stop=(c == C - 2))
            elif q < 3:
                for c in range(C - 1):
                    if c >= 1:
                        unit_boundary()
                    emit_single(q, c, 0, QW, halves, start=(c == 0), stop=False)
                unit_boundary()
                emit_single(q, C - 1, 0, QW, halves, start=False, stop=True)
            else:
                # last quarter: class 18 last, in two 512 chunks, so the
                # end-drain after the final (small) DMA is short
                for c in range(C - 1):
                    if c >= 1:
                        unit_boundary()
                    emit_single(q, c, 0, QW, halves, start=(c == 0), stop=False)
                emit_single(q, C - 1, 0, HQW, halves, start=False, stop=True)
                emit_single(q, C - 1, HQW, HQW, halves, start=False, stop=True)

            if q < 3:
                pending_ops = tail_ops(q, 0, *halves[0]) + tail_ops(q, 1, *halves[1])
            else:
                # final tails inline, interleaved a/b for minimal latency
                ta = tail_ops(q, 0, halves[0][0], halves[0][1])
                tb = tail_ops(q, 1, halves[1][0], halves[1][1])
                for op in (ta[0], ta[1], tb[0], tb[1], ta[2], ta[3], tb[2], tb[3]):
                    op()

        nc.sync.dma_start(out=out[:], in_=stats[:])

    if not nc.is_finalized():
        nc.finalize()

    return nc


_NC_CACHE = None


def make_in_maps(predict: np.ndarray, target: np.ndarray):
    import ml_dtypes

    predict = np.ascontiguousarray(predict, dtype=np.float32)
    target_bf = np.ascontiguousarray(target, dtype=np.int32).astype(ml_dtypes.bfloat16)
    idn = np.eye(P, dtype=np.float32).astype(ml_dtypes.bfloat16)

    in_maps = []
    for k in range(N_CORES):
        in_maps.append(
            {
                "predict": predict[k].reshape(C, PIX),
                "target": target_bf[k].reshape(P, FCOLS),
                "idn": idn,
            }
        )
    return in_maps


def combine_host(results, target: np.ndarray) -> np.float32:
    tot = np.float64(0.0)
    s_all = np.float64(0.0)
    s_pos = np.float64(0.0)
    for k in range(N_CORES):
        st = results[k]["out"].reshape(P, NSTAT).astype(np.float64)
        s_all += -np.sum(st[:, COL_LNR : COL_LNR + 8])
        s_pos += -np.sum(st[:, COL_POSLNR : COL_POSLNR + 8])
        tot += PIX
    pos = np.float64(np.count_nonzero(target))
    neg = tot - pos
    s_neg = s_all - s_pos
    loss = ((neg / tot) * s_pos + (pos / tot) * s_neg) / (tot * C)
    return np.float32(loss)


def kernel(predict: np.ndarray, target: np.ndarray) -> np.ndarray:
    global _NC_CACHE
    if _NC_CACHE is None:
        _NC_CACHE = build_kernel()
    nc = _NC_CACHE

    in_maps = make_in_maps(predict, target)
    res = run_bass_kernel_spmd(nc, in_maps, list(range(N_CORES)))
    return combine_host(res.results, target)
